# revision 3
# baseline (speedup 1.0000x reference)
"""Trainium2 Bass kernel for nn_Decoder (ragged sinusoidal-query decoder).

Math (per sample b):
    n      = z[b, 64]                       count in [1, 16]
    zf     = z[b, :64] viewed as [8, 8]
    query  = pos_enc(16, 128) @ Wq_w.T      [16, 8]   (input-dependent, tiny)
    x[b]   = (query @ zf @ map_w.T + map_b) * mask    [16, 128]
    mask   = arange(16) < n

The whole thing collapses to ONE matmul per sample:
    x[b].flatten() = z_aug[b] @ W_aug  masked per (b, p)
where W_aug[65, 2048] = [kron-ish combine of query and map_w ; bias row]
and z_aug[b] = [zf (64) | 1.0]. The mask multiplies blocks of 128
consecutive output columns, i.e. mask_flat[b, j] = (j < 128 * n[b]).

Device kernel per core (8192 rows, data-parallel over 8 cores):
  - DMA z tile [128, 66]  (cols: 64 feat | ones | 8192*n - 32)
  - TensorE transpose -> zT [65, 128] (ScalarE evicts PSUM->SBUF)
  - 4x TensorE matmul [65,128]^T @ [65,512] -> PSUM [128, 2048]
  - ScalarE: mask = Sigmoid(-64 * iota + (8192*n - 32))  -> exact 0/1
  - VectorE: out_sbuf = psum * mask   (eviction fused with masking)
  - DMA out tile [128, 2048]
"""

import numpy as np

B = 65536
DIM = 128
WD = 8
P = 16
N_CORES = 8
ROWS = B // N_CORES          # 8192 rows per core
RTILE = 128                  # rows per on-chip tile
NTILES = ROWS // RTILE       # 64
KAUG = 65                    # 64 features + ones column
NOUT = P * DIM               # 2048 output columns

_CACHE = {}


def _build_nc():
    import concourse.mybir as mybir
    import concourse.tile as tile
    from concourse import bacc
    from concourse.masks import make_identity
    from contextlib import ExitStack

    f32 = mybir.dt.float32
    nc = bacc.Bacc(None, target_bir_lowering=False)

    z = nc.declare_dram_parameter("z", [ROWS, 66], f32, isOutput=False)
    w = nc.declare_dram_parameter("w", [KAUG, NOUT], f32, isOutput=False)
    iota = nc.declare_dram_parameter("iota", [1, NOUT], f32, isOutput=False)
    out = nc.declare_dram_parameter("out", [ROWS, NOUT], f32, isOutput=True)

    with tile.TileContext(nc) as tc, ExitStack() as ctx:
        singles = ctx.enter_context(tc.tile_pool(name="singles", bufs=1))
        zpool = ctx.enter_context(tc.tile_pool(name="zpool", bufs=4))
        ztps = ctx.enter_context(tc.tile_pool(name="ztps", bufs=2, space="PSUM"))
        ztsb = ctx.enter_context(tc.tile_pool(name="ztsb", bufs=3))
        mmps = ctx.enter_context(tc.tile_pool(name="mmps", bufs=3, space="PSUM"))
        maskp = ctx.enter_context(tc.tile_pool(name="maskp", bufs=3))
        outp = ctx.enter_context(tc.tile_pool(name="outp", bufs=4))

        identity = singles.tile([128, 128], f32)
        make_identity(nc, identity)

        w_sb = singles.tile([KAUG, NOUT], f32)
        nc.sync.dma_start(out=w_sb, in_=w[:, :])

        iota_sb = singles.tile([128, NOUT], f32)
        nc.sync.dma_start(out=iota_sb, in_=iota[0:1, :].to_broadcast([128, NOUT]))

        for i in range(NTILES):
            r0 = i * RTILE
            z_t = zpool.tile([RTILE, 66], f32)
            nc.sync.dma_start(out=z_t, in_=z[r0 : r0 + RTILE, :])

            zt_ps = ztps.tile([KAUG, RTILE], f32)
            nc.tensor.transpose(zt_ps, z_t[:, 0:KAUG], identity)
            zt_sb = ztsb.tile([KAUG, RTILE], f32)
            nc.scalar.copy(zt_sb, zt_ps)

            mask_t = maskp.tile([RTILE, NOUT], f32)
            nc.scalar.activation(
                out=mask_t,
                in_=iota_sb,
                func=mybir.ActivationFunctionType.Sigmoid,
                bias=z_t[:, 65:66],
                scale=-64.0,
            )

            x_t = outp.tile([RTILE, NOUT], f32)
            for q in range(2):
                mm_ps = mmps.tile([RTILE, 1024], f32)
                for h in range(2):
                    c0 = q * 1024 + h * 512
                    nc.tensor.matmul(
                        out=mm_ps[:, h * 512 : (h + 1) * 512],
                        lhsT=zt_sb,
                        rhs=w_sb[:, c0 : c0 + 512],
                        start=True,
                        stop=True,
                    )
                nc.vector.tensor_mul(
                    x_t[:, q * 1024 : (q + 1) * 1024],
                    mm_ps,
                    mask_t[:, q * 1024 : (q + 1) * 1024],
                )

            nc.sync.dma_start(out=out[r0 : r0 + RTILE, :], in_=x_t)

    nc.finalize()
    return nc


def _sinusoid_pos_enc(length, dim):
    pos = np.arange(length, dtype=np.float32)[:, None]
    i = np.arange(0, dim, 2, dtype=np.float32)
    inv_freq = np.exp(-np.log(10000.0) * i / dim).astype(np.float32)
    ang = pos * inv_freq
    pe = np.zeros((length, dim), dtype=np.float32)
    pe[:, 0::2] = np.sin(ang)
    pe[:, 1::2] = np.cos(ang)
    return pe


def kernel(z, Wq_w, map_w, map_b):
    from concourse.bass_utils import run_bass_kernel_spmd

    z = np.asarray(z, dtype=np.float32)
    Wq_w = np.asarray(Wq_w, dtype=np.float32)
    map_w = np.asarray(map_w, dtype=np.float32)
    map_b = np.asarray(map_b, dtype=np.float32)

    n = z[:, 64].astype(np.int32)                       # [B], 1..16
    zf = z[:, :64]

    # z_aug: [B, 66] = [features | ones | 8192*n - 32]
    z_aug = np.empty((B, 66), dtype=np.float32)
    z_aug[:, :64] = zf
    z_aug[:, 64] = 1.0
    z_aug[:, 65] = (8192.0 * n - 32.0).astype(np.float32)

    # W_aug: [65, 2048]; row (k*8+d), col (p*128+o) = query[p,k]*map_w[o,d]
    query = _sinusoid_pos_enc(P, DIM) @ Wq_w.T          # [16, 8]
    w_comb = np.einsum("pk,od->kdpo", query.astype(np.float64),
                       map_w.astype(np.float64)).reshape(64, NOUT)
    w_aug = np.empty((KAUG, NOUT), dtype=np.float32)
    w_aug[:64] = w_comb.astype(np.float32)
    w_aug[64] = np.tile(map_b, P)

    iota_row = np.arange(NOUT, dtype=np.float32).reshape(1, NOUT)

    if "nc" not in _CACHE:
        _CACHE["nc"] = _build_nc()
    nc = _CACHE["nc"]

    in_maps = [
        {
            "z": np.ascontiguousarray(z_aug[c * ROWS : (c + 1) * ROWS]),
            "w": w_aug,
            "iota": iota_row,
        }
        for c in range(N_CORES)
    ]
    res = run_bass_kernel_spmd(nc, in_maps, core_ids=list(range(N_CORES)))
    x = np.concatenate([r["out"] for r in res.results], axis=0).reshape(B, P, DIM)

    mask = np.arange(P, dtype=np.int32)[None, :] < n[:, None]
    batch = np.ascontiguousarray(
        np.broadcast_to(np.arange(B, dtype=np.int32)[:, None], (B, P))
    )
    return x, mask, batch


# revision 4
# speedup vs baseline: 1.2180x; 1.2180x over previous
"""Trainium2 Bass kernel for nn_Decoder (ragged sinusoidal-query decoder).

Math (per sample b):
    n      = z[b, 64]                       count in [1, 16]
    zf     = z[b, :64] viewed as [8, 8]
    query  = pos_enc(16, 128) @ Wq_w.T      [16, 8]
    x[b]   = (query @ zf @ map_w.T + map_b) * mask    [16, 128]
    mask   = arange(16) < n

The whole thing collapses to ONE matmul per sample:
    x[b].flatten() = z_aug[b] @ W_aug  masked per (b, p)
where W_aug[65, 2048] combines query and map_w (plus a bias row hit by a
ones-column in z_aug), and mask_flat[b, j] = (j < 128 * n[b]).

Device kernel per core (8192 rows, data-parallel over 8 cores):
  - z is pre-transposed and hi/lo-split to bf16 on the host, so lhsT
    tiles [65, 128] DMA straight in; no on-chip transpose.
  - TensorE: 12 bf16 matmuls per 128-row tile accumulating
    z_hi@W_hi + z_hi@W_lo + z_lo@W_hi  (~fp32 precision, bf16 speed)
  - ScalarE: mask = Sigmoid(-64*iota + (8192*n - 32)) -> exact 0/1
  - VectorE: out_sbuf = psum * mask  (eviction fused with masking)
  - DMA out tile [128, 2048] fp32
"""

import numpy as np

B = 65536
DIM = 128
WD = 8
P = 16
N_CORES = 8
ROWS = B // N_CORES          # 8192 rows per core
RTILE = 128                  # rows per on-chip tile
NTILES = ROWS // RTILE       # 64
KAUG = 65                    # 64 features + ones row
NOUT = P * DIM               # 2048 output columns

_CACHE = {}


def _build_nc():
    import concourse.mybir as mybir
    import concourse.tile as tile
    from concourse import bacc
    from contextlib import ExitStack

    f32 = mybir.dt.float32
    bf16 = mybir.dt.bfloat16
    nc = bacc.Bacc(None, target_bir_lowering=False)

    zt_hi = nc.declare_dram_parameter("zt_hi", [KAUG, ROWS], bf16, isOutput=False)
    zt_lo = nc.declare_dram_parameter("zt_lo", [KAUG, ROWS], bf16, isOutput=False)
    w_hi = nc.declare_dram_parameter("w_hi", [KAUG, NOUT], bf16, isOutput=False)
    w_lo = nc.declare_dram_parameter("w_lo", [KAUG, NOUT], bf16, isOutput=False)
    nsc = nc.declare_dram_parameter("nsc", [RTILE, NTILES], f32, isOutput=False)
    iota = nc.declare_dram_parameter("iota", [1, NOUT], f32, isOutput=False)
    out = nc.declare_dram_parameter("out", [ROWS, NOUT], f32, isOutput=True)

    with tile.TileContext(nc) as tc, ExitStack() as ctx:
        singles = ctx.enter_context(tc.tile_pool(name="singles", bufs=1))
        zpool = ctx.enter_context(tc.tile_pool(name="zpool", bufs=6))
        mmps = ctx.enter_context(tc.tile_pool(name="mmps", bufs=4, space="PSUM"))
        maskp = ctx.enter_context(tc.tile_pool(name="maskp", bufs=3))
        outp = ctx.enter_context(tc.tile_pool(name="outp", bufs=4))

        wh_sb = singles.tile([KAUG, NOUT], bf16)
        nc.sync.dma_start(out=wh_sb, in_=w_hi[:, :])
        wl_sb = singles.tile([KAUG, NOUT], bf16)
        nc.sync.dma_start(out=wl_sb, in_=w_lo[:, :])
        nsc_sb = singles.tile([RTILE, NTILES], f32)
        nc.sync.dma_start(out=nsc_sb, in_=nsc[:, :])
        iota_sb = singles.tile([128, NOUT], f32)
        nc.sync.dma_start(out=iota_sb, in_=iota[0:1, :].to_broadcast([128, NOUT]))

        for i in range(NTILES):
            r0 = i * RTILE
            zh_t = zpool.tile([KAUG, RTILE], bf16, tag="zh")
            nc.sync.dma_start(out=zh_t, in_=zt_hi[:, r0 : r0 + RTILE])
            zl_t = zpool.tile([KAUG, RTILE], bf16, tag="zl")
            nc.sync.dma_start(out=zl_t, in_=zt_lo[:, r0 : r0 + RTILE])

            mask_t = maskp.tile([RTILE, NOUT], f32)
            nc.scalar.activation(
                out=mask_t,
                in_=iota_sb,
                func=mybir.ActivationFunctionType.Sigmoid,
                bias=nsc_sb[:, i : i + 1],
                scale=-64.0,
            )

            x_t = outp.tile([RTILE, NOUT], f32)
            for q in range(2):
                mm_ps = mmps.tile([RTILE, 1024], f32)
                for h in range(2):
                    c0 = q * 1024 + h * 512
                    pslice = mm_ps[:, h * 512 : (h + 1) * 512]
                    nc.tensor.matmul(
                        out=pslice, lhsT=zh_t, rhs=wh_sb[:, c0 : c0 + 512],
                        start=True, stop=False,
                    )
                    nc.tensor.matmul(
                        out=pslice, lhsT=zh_t, rhs=wl_sb[:, c0 : c0 + 512],
                        start=False, stop=False,
                    )
                    nc.tensor.matmul(
                        out=pslice, lhsT=zl_t, rhs=wh_sb[:, c0 : c0 + 512],
                        start=False, stop=True,
                    )
                nc.vector.tensor_mul(
                    x_t[:, q * 1024 : (q + 1) * 1024],
                    mm_ps,
                    mask_t[:, q * 1024 : (q + 1) * 1024],
                )

            nc.sync.dma_start(out=out[r0 : r0 + RTILE, :], in_=x_t)

    nc.finalize()
    return nc


def _sinusoid_pos_enc(length, dim):
    pos = np.arange(length, dtype=np.float32)[:, None]
    i = np.arange(0, dim, 2, dtype=np.float32)
    inv_freq = np.exp(-np.log(10000.0) * i / dim).astype(np.float32)
    ang = pos * inv_freq
    pe = np.zeros((length, dim), dtype=np.float32)
    pe[:, 0::2] = np.sin(ang)
    pe[:, 1::2] = np.cos(ang)
    return pe


def _host_prep(z, Wq_w, map_w, map_b):
    import ml_dtypes

    bf16 = ml_dtypes.bfloat16
    n = z[:, 64].astype(np.int32)                       # [B], 1..16
    # z_aug^T: [65, B] = [features ; ones] then hi/lo split to bf16
    zt = np.empty((KAUG, B), dtype=np.float32)
    zt[:64] = z[:, :64].T
    zt[64] = 1.0
    zt_hi = zt.astype(bf16)
    zt_lo = (zt - zt_hi.astype(np.float32)).astype(bf16)

    # W_aug: [65, 2048]; row (k*8+d), col (p*128+o) = query[p,k]*map_w[o,d]
    query = _sinusoid_pos_enc(P, DIM) @ Wq_w.T          # [16, 8]
    w_comb = np.einsum("pk,od->kdpo", query.astype(np.float64),
                       map_w.astype(np.float64)).reshape(64, NOUT)
    w_aug = np.empty((KAUG, NOUT), dtype=np.float32)
    w_aug[:64] = w_comb.astype(np.float32)
    w_aug[64] = np.tile(map_b, P)
    w_hi = w_aug.astype(bf16)
    w_lo = (w_aug - w_hi.astype(np.float32)).astype(bf16)

    # per-core, per-tile layout of the mask threshold 8192*n - 32
    nscaled = (8192.0 * n - 32.0).astype(np.float32)
    iota_row = np.arange(NOUT, dtype=np.float32).reshape(1, NOUT)

    in_maps = []
    for c in range(N_CORES):
        s = slice(c * ROWS, (c + 1) * ROWS)
        in_maps.append({
            "zt_hi": np.ascontiguousarray(zt_hi[:, s]),
            "zt_lo": np.ascontiguousarray(zt_lo[:, s]),
            "w_hi": w_hi,
            "w_lo": w_lo,
            "nsc": np.ascontiguousarray(
                nscaled[s].reshape(NTILES, RTILE).T),
            "iota": iota_row,
        })
    return in_maps, n


def kernel(z, Wq_w, map_w, map_b):
    from concourse.bass_utils import run_bass_kernel_spmd

    z = np.asarray(z, dtype=np.float32)
    Wq_w = np.asarray(Wq_w, dtype=np.float32)
    map_w = np.asarray(map_w, dtype=np.float32)
    map_b = np.asarray(map_b, dtype=np.float32)

    in_maps, n = _host_prep(z, Wq_w, map_w, map_b)

    if "nc" not in _CACHE:
        _CACHE["nc"] = _build_nc()
    nc = _CACHE["nc"]

    res = run_bass_kernel_spmd(nc, in_maps, core_ids=list(range(N_CORES)))
    x = np.concatenate([r["out"] for r in res.results], axis=0).reshape(B, P, DIM)

    mask = np.arange(P, dtype=np.int32)[None, :] < n[:, None]
    batch = np.ascontiguousarray(
        np.broadcast_to(np.arange(B, dtype=np.int32)[:, None], (B, P))
    )
    return x, mask, batch


# revision 7
# speedup vs baseline: 2.1692x; 1.7809x over previous
"""Trainium2 Bass kernel for nn_Decoder (ragged sinusoidal-query decoder).

Math (per sample b):
    n      = z[b, 64]                       count in [1, 16]
    zf     = z[b, :64] viewed as [8, 8]
    query  = pos_enc(16, 128) @ Wq_w.T      [16, 8]
    x[b]   = (query @ zf @ map_w.T + map_b) * mask    [16, 128]
    mask   = arange(16) < n

The whole thing collapses to ONE matmul per sample:
    x[b].flatten() = z_aug[b] @ W_aug  masked per (b, p)
where W_aug[65, 2048] combines query and map_w (plus a bias row hit by a
ones-column in z_aug), and mask_flat[b, j] = (j < 128 * n[b]).

For near-fp32 accuracy at bf16 PE speed the product is computed as
    z_hi@W_hi + z_lo@W_hi + z_hi@W_lo      (hi/lo bf16 split, host-prepped)
and the three terms are packed into TWO matmuls by row concatenation:
    A: [z_hi(64) ; 1 ; z_lo[0:31]]  (K=96)  @  [W_hi ; bias_hi ; W_hi[0:31]]
    B: [z_lo[31:64] ; 1 ; z_hi(64)] (K=98)  @  [W_hi[31:64] ; bias_lo ; W_lo]
(K <= 64 runs the PE at half clock on trn2, so K=96/98 also dodges that.)

Device kernel per core (8192 rows, data-parallel over 8 cores):
  - z is pre-transposed/packed on the host; lhsT tiles DMA straight in
    on the ACT HWDGE ring (separate from the output ring, no starvation).
  - TensorE: 8 bf16 matmuls per 128-row tile (2 accumulating per 512-col
    PSUM bank).
  - ScalarE: mask = Sigmoid(-64*iota + (8192*n - 32)) -> exact 0/1
  - VectorE: out_sbuf = psum * mask  (eviction fused with masking)
  - DMA out tile [128, 2048] fp32 on the sync HWDGE ring.
"""

import numpy as np

B = 65536
DIM = 128
WD = 8
P = 16
N_CORES = 8
ROWS = B // N_CORES          # 8192 rows per core
RTILE = 128                  # rows per on-chip tile
NTILES = ROWS // RTILE       # 64
KA = 96                      # pass A contraction
KB = 98                      # pass B contraction
NOUT = P * DIM               # 2048 output columns

_CACHE = {}


def _build_nc():
    import concourse.mybir as mybir
    import concourse.tile as tile
    from concourse import bacc
    from contextlib import ExitStack

    f32 = mybir.dt.float32
    bf16 = mybir.dt.bfloat16
    nc = bacc.Bacc(None, target_bir_lowering=False)

    za = nc.declare_dram_parameter("za", [KA, ROWS], bf16, isOutput=False)
    zb = nc.declare_dram_parameter("zb", [KB, ROWS], bf16, isOutput=False)
    wa = nc.declare_dram_parameter("wa", [KA, NOUT], bf16, isOutput=False)
    wb = nc.declare_dram_parameter("wb", [KB, NOUT], bf16, isOutput=False)
    nsc = nc.declare_dram_parameter("nsc", [RTILE, NTILES], f32, isOutput=False)
    out = nc.declare_dram_parameter("out", [ROWS, NOUT], f32, isOutput=True)

    with tile.TileContext(nc) as tc, ExitStack() as ctx:
        singles = ctx.enter_context(tc.tile_pool(name="singles", bufs=1))
        zpool = ctx.enter_context(tc.tile_pool(name="zpool", bufs=12))
        mmps = ctx.enter_context(tc.tile_pool(name="mmps", bufs=4, space="PSUM"))
        maskp = ctx.enter_context(tc.tile_pool(name="maskp", bufs=3))
        outp = ctx.enter_context(tc.tile_pool(name="outp", bufs=4))

        wa_sb = singles.tile([KA, NOUT], bf16)
        nc.scalar.dma_start(out=wa_sb, in_=wa[:, :])
        wb_sb = singles.tile([KB, NOUT], bf16)
        nc.scalar.dma_start(out=wb_sb, in_=wb[:, :])
        nsc_sb = singles.tile([RTILE, NTILES], f32)
        nc.scalar.dma_start(out=nsc_sb, in_=nsc[:, :])
        iota_sb = singles.tile([128, NOUT], f32)
        nc.gpsimd.iota(iota_sb[:, :], [[1, NOUT]], channel_multiplier=0,
                       allow_small_or_imprecise_dtypes=True)

        for i in range(NTILES):
            r0 = i * RTILE
            za_t = zpool.tile([KA, RTILE], bf16, tag="za")
            nc.scalar.dma_start(out=za_t, in_=za[:, r0 : r0 + RTILE])
            zb_t = zpool.tile([KB, RTILE], bf16, tag="zb")
            nc.scalar.dma_start(out=zb_t, in_=zb[:, r0 : r0 + RTILE])

            mask_t = maskp.tile([RTILE, NOUT], f32)
            nc.scalar.activation(
                out=mask_t,
                in_=iota_sb,
                func=mybir.ActivationFunctionType.Sigmoid,
                bias=nsc_sb[:, i : i + 1],
                scale=-64.0,
            )

            x_t = outp.tile([RTILE, NOUT], f32)
            for q in range(2):
                mm_ps = mmps.tile([RTILE, 1024], f32)
                for h in range(2):
                    c0 = q * 1024 + h * 512
                    pslice = mm_ps[:, h * 512 : (h + 1) * 512]
                    nc.tensor.matmul(
                        out=pslice, lhsT=za_t, rhs=wa_sb[:, c0 : c0 + 512],
                        start=True, stop=False,
                    )
                    nc.tensor.matmul(
                        out=pslice, lhsT=zb_t, rhs=wb_sb[:, c0 : c0 + 512],
                        start=False, stop=True,
                    )
                nc.vector.tensor_mul(
                    x_t[:, q * 1024 : (q + 1) * 1024],
                    mm_ps,
                    mask_t[:, q * 1024 : (q + 1) * 1024],
                )

            nc.sync.dma_start(out=out[r0 : r0 + RTILE, :], in_=x_t)

    nc.finalize()
    return nc


def _sinusoid_pos_enc(length, dim):
    pos = np.arange(length, dtype=np.float32)[:, None]
    i = np.arange(0, dim, 2, dtype=np.float32)
    inv_freq = np.exp(-np.log(10000.0) * i / dim).astype(np.float32)
    ang = pos * inv_freq
    pe = np.zeros((length, dim), dtype=np.float32)
    pe[:, 0::2] = np.sin(ang)
    pe[:, 1::2] = np.cos(ang)
    return pe


def _host_prep(z, Wq_w, map_w, map_b):
    import ml_dtypes

    bf16 = ml_dtypes.bfloat16
    n = z[:, 64].astype(np.int32)                       # [B], 1..16

    zf_t = np.ascontiguousarray(z[:, :64].T)            # [64, B] fp32
    zf_hi = zf_t.astype(bf16)
    zf_lo = (zf_t - zf_hi.astype(np.float32)).astype(bf16)

    # pass A lhsT rows: [z_hi(64) ; ones ; z_lo[0:31]]          (96)
    za = np.empty((KA, B), dtype=bf16)
    za[:64] = zf_hi
    za[64] = np.float32(1.0)
    za[65:] = zf_lo[0:31]
    # pass B lhsT rows: [z_lo[31:64] ; ones ; z_hi(64)]          (98)
    zb = np.empty((KB, B), dtype=bf16)
    zb[0:33] = zf_lo[31:64]
    zb[33] = np.float32(1.0)
    zb[34:] = zf_hi

    # W_comb: [64, 2048]; row (k*8+d), col (p*128+o) = query[p,k]*map_w[o,d]
    query = _sinusoid_pos_enc(P, DIM) @ Wq_w.T          # [16, 8]
    w_comb = np.einsum("pk,od->kdpo", query.astype(np.float64),
                       map_w.astype(np.float64)).reshape(64, NOUT).astype(np.float32)
    w_hi = w_comb.astype(bf16)
    w_lo = (w_comb - w_hi.astype(np.float32)).astype(bf16)
    bias = np.tile(map_b, P).astype(np.float32)         # [2048]
    bias_hi = bias.astype(bf16)
    bias_lo = (bias - bias_hi.astype(np.float32)).astype(bf16)

    wa = np.empty((KA, NOUT), dtype=bf16)
    wa[:64] = w_hi
    wa[64] = bias_hi
    wa[65:] = w_hi[0:31]
    wb = np.empty((KB, NOUT), dtype=bf16)
    wb[0:33] = w_hi[31:64]
    wb[33] = bias_lo
    wb[34:] = w_lo

    nscaled = (8192.0 * n - 32.0).astype(np.float32)

    in_maps = []
    for c in range(N_CORES):
        s = slice(c * ROWS, (c + 1) * ROWS)
        in_maps.append({
            "za": np.ascontiguousarray(za[:, s]),
            "zb": np.ascontiguousarray(zb[:, s]),
            "wa": wa,
            "wb": wb,
            "nsc": np.ascontiguousarray(
                nscaled[s].reshape(NTILES, RTILE).T),
        })
    return in_maps, n


def kernel(z, Wq_w, map_w, map_b):
    from concourse.bass_utils import run_bass_kernel_spmd

    z = np.asarray(z, dtype=np.float32)
    Wq_w = np.asarray(Wq_w, dtype=np.float32)
    map_w = np.asarray(map_w, dtype=np.float32)
    map_b = np.asarray(map_b, dtype=np.float32)

    in_maps, n = _host_prep(z, Wq_w, map_w, map_b)

    if "nc" not in _CACHE:
        _CACHE["nc"] = _build_nc()
    nc = _CACHE["nc"]

    res = run_bass_kernel_spmd(nc, in_maps, core_ids=list(range(N_CORES)))
    x = np.concatenate([r["out"] for r in res.results], axis=0).reshape(B, P, DIM)

    mask = np.arange(P, dtype=np.int32)[None, :] < n[:, None]
    batch = np.ascontiguousarray(
        np.broadcast_to(np.arange(B, dtype=np.int32)[:, None], (B, P))
    )
    return x, mask, batch
